# revision 14
# baseline (speedup 1.0000x reference)
"""Causal self-attention (B=2, T=4096, C=768, H=12, D=64) on 8 trn2 cores.

Sharding: core c handles batch b = c//4 and heads [3g, 3g+3), g = c%4.
Each core computes a (4096, 768) partial of y = attn_out @ w_out restricted
to its 3 heads' rows of w_out; the host sums the 4 partials per batch.

v1 layout (vs baseline): x arrives host-transposed (xT [C, T]) so no PE
transposes are needed; V is projected token-major directly (stationary =
xT chunk, moving = w_v), QK uses feature-major Q^T/K^T from 3 projection
slots [q0|q1], [k0|k1], [q2|k2] (k2 partition-shifted via SBUF DMA).
Causal masking touches only the [128,128] triangle block per diagonal
k-tile (gpsimd affine_select); the fully-masked columns are skipped by
column-restricted exp + PV accumulation. PV of group g is issued after
the QK+exp of group g+1 so the scalar engine (exp is the global floor,
~1.57us per k-tile) never starves.

Math per head (no max-subtraction softmax; scores are O(8) so exp is safe):
  S^T[k, q] = (K Q^T)[k, q] / 8     computed k-on-partitions (f32r matmuls)
  E = exp(S^T) * causal_mask
  [Y^T; l] = [V | 1]^T E            PV matmul with a ones column -> row 64 = l
  out += (Y^T / l).T @ W_o[head rows]
"""

import os
import numpy as np
import ml_dtypes
from contextlib import ExitStack

import concourse.bass as bass
import concourse.tile as tile
from concourse import bacc, mybir
from concourse.bass_utils import run_bass_kernel_spmd

F32 = mybir.dt.float32
BF16 = mybir.dt.bfloat16

B, T, C, H, D = 2, 4096, 768, 12, 64
HPC = 3            # heads per core
NS = 8             # strips
SW = 512           # strip width (q)
KT = 128           # k tile
NKT = T // KT      # 32 k tiles
KG = 8             # k tiles per PV accumulation group





def build_program():
    nc = bacc.Bacc("TRN2", target_bir_lowering=False, debug=False, num_devices=8)

    x_d = nc.dram_tensor("xT", [C, T], BF16, kind="ExternalInput").ap()
    wqk_d = nc.dram_tensor("wqk", [C, 384], BF16, kind="ExternalInput").ap()
    wv_d = nc.dram_tensor("wv", [C, 192], BF16, kind="ExternalInput").ap()
    woA_d = nc.dram_tensor("woA", [128, C], BF16, kind="ExternalInput").ap()
    woB_d = nc.dram_tensor("woB", [64, C], BF16, kind="ExternalInput").ap()
    y_d = nc.dram_tensor("y", [T, C], F32, kind="ExternalOutput").ap()

    with tile.TileContext(nc) as tc, ExitStack() as ctx:
        kernel_body(tc, ctx, x_d, wqk_d, wv_d, woA_d, woB_d, y_d)
    nc.compile()
    return nc


def kernel_body(tc, ctx, x_d, wqk_d, wv_d, woA_d, woB_d, y_d):
    nc = tc.nc
    EXP = mybir.ActivationFunctionType.Exp
    k_diag = int(os.environ.get("KDIAG", "1"))   # col-restricted diag PV
    k_pb = int(os.environ.get("KPB", "0"))       # gpsimd partition_broadcast
    k_rf = int(os.environ.get("KRF", "0"))       # reciprocal_approx_fast
    k_warm = int(os.environ.get("KWARM", "1"))   # PE warmup matmuls
    dram_pool = ctx.enter_context(tc.tile_pool(name="dram", bufs=1, space="DRAM"))
    scratch_d = dram_pool.tile([NS, HPC, SW], F32, name="scratch")

    singles = ctx.enter_context(tc.tile_pool(name="singles", bufs=1))
    xt_pool = ctx.enter_context(tc.tile_pool(name="xt_pool", bufs=3))
    qq_pool = ctx.enter_context(tc.tile_pool(name="qq_pool", bufs=2))
    es_pool = ctx.enter_context(tc.tile_pool(name="es_pool", bufs=20))
    ya_pool = ctx.enter_context(tc.tile_pool(name="ya_pool", bufs=2))
    rl_pool = ctx.enter_context(tc.tile_pool(name="rl_pool", bufs=2))
    yst_pool = ctx.enter_context(tc.tile_pool(name="yst_pool", bufs=2))
    out_pool = ctx.enter_context(tc.tile_pool(name="out_pool", bufs=2))
    ps_s = ctx.enter_context(tc.tile_pool(name="ps_s", bufs=2, space="PSUM"))
    ps_y = ctx.enter_context(tc.tile_pool(name="ps_y", bufs=2, space="PSUM"))

    # ---- PE warmup: junk matmuls during the initial DMA wait keep HAM hot ----
    junk = singles.tile([128, 128], BF16)
    nc.vector.memset(junk, 0.015625)
    if k_warm:
        psj = ps_s.tile([128, SW], F32, name="ps_warm", tag="S")
        for w in range(48):
            nc.tensor.matmul(psj[:, 0:128], (junk), (junk),
                             start=True, stop=True)

    # ---- weights (xT strip 0 + wqk first: they gate the first QK) ----
    xT_tiles = [None] * NS
    xt0 = []
    for kc in range(6):
        xt = xt_pool.tile([128, SW], BF16, name=f"xT_0_{kc}", tag=f"xT{kc}")
        nc.sync.dma_start(xt, x_d[kc * 128:(kc + 1) * 128, 0:SW])
        xt0.append(xt)
    xT_tiles[0] = xt0
    wqk_sb = []
    for kc in range(6):
        wt = singles.tile([128, 384], BF16, name=f"wqk_sb{kc}")
        nc.sync.dma_start(wt, wqk_d[kc * 128:(kc + 1) * 128, :])
        wqk_sb.append(wt)
    wv_sb = []
    for kc in range(6):
        wt = singles.tile([128, 192], BF16, name=f"wv_sb{kc}")
        nc.sync.dma_start(wt, wv_d[kc * 128:(kc + 1) * 128, :])
        wv_sb.append(wt)
    woA = singles.tile([128, C], BF16)
    nc.sync.dma_start(woA, woA_d)
    woB = singles.tile([64, C], BF16)
    nc.sync.dma_start(woB, woB_d)

    # resident K storage: KK[s] = [k0|k1] feature-major, K2c[s] = k2 at p0:64
    KK = [singles.tile([128, SW], BF16, name=f"KK{s}") for s in range(NS)]
    K2c = [singles.tile([64, SW], BF16, name=f"K2c{s}") for s in range(NS)]

    # token-major V with ones column per head, all 32 k-tiles
    vtm = [singles.tile([128, NKT, D + 1], BF16, name=f"vtm{h}") for h in range(HPC)]
    ones_col = singles.tile([128, NKT], BF16)
    nc.vector.memset(ones_col, 1.0)
    for h in range(HPC):
        nc.vector.tensor_copy(vtm[h][:, :, D:D + 1], ones_col.unsqueeze(2))

    qq_tiles = [None] * NS

    # ---------------- Phase A for one strip (chunk generator) ----------------
    def phase_a_dma(s):
        xT = []
        for kc in range(6):
            xt = xt_pool.tile([128, SW], BF16, name=f"xT_{s}_{kc}", tag=f"xT{kc}")
            nc.sync.dma_start(xt, x_d[kc * 128:(kc + 1) * 128,
                                      s * SW:(s + 1) * SW])
            xT.append(xt)
        xT_tiles[s] = xT

    def phase_a_proj(s):
        xT = xT_tiles[s]
        # projection slots: [q0|q1], [k0|k1], [q2|k2]
        psp = ps_s.tile([128, 3, SW], F32, name=f"ps_pj_{s}", tag="S")
        for u in range(3):
            for kc in range(6):
                nc.tensor.matmul(psp[:, u, :],
                                 (wqk_sb[kc][:, u * 128:(u + 1) * 128]),
                                 (xT[kc]), start=(kc == 0), stop=(kc == 5))
            yield
        qq = qq_pool.tile([128, SW], BF16, name=f"qq_{s}", tag="qq")
        tmp = qq_pool.tile([128, SW], BF16, name=f"q2k2_{s}", tag="q2k2")
        nc.vector.tensor_copy(qq, psp[:, 0, :])
        nc.vector.tensor_copy(KK[s], psp[:, 1, :])
        nc.vector.tensor_copy(tmp, psp[:, 2, :])
        qq_tiles[s] = (qq, tmp)
        # k2 partition shift p64:128 -> p0:64 (SBUF->SBUF DMA)
        nc.gpsimd.dma_start(K2c[s], tmp[64:128, :])
        yield

    def phase_a_v(s):
        # V token-major: stationary = xT chunk slice, moving = w_v [128, 192]
        # (pure filler: vtm k-tiles of strip s are first read by strip s's
        # diagonal PV unit, which issues at the end of strip s)
        xT = xT_tiles[s]
        for tt in range(4):
            psv = ps_s.tile([128, 192], F32, name=f"ps_v_{s}_{tt}", tag="S")
            for kc in range(6):
                nc.tensor.matmul(psv,
                                 (xT[kc][:, tt * 128:(tt + 1) * 128]),
                                 (wv_sb[kc]), start=(kc == 0), stop=(kc == 5))
            kt = 4 * s + tt
            for h in range(HPC):
                nc.vector.tensor_copy(vtm[h][:, kt, 0:D],
                                      psv[:, h * 64:(h + 1) * 64])
            yield

    # -------- Phase B: one continuous pipeline over all 144 k-tiles --------
    # Per tick (one k-tile): QK triplet + exp + ~3 PV matmuls from the unit
    # queue (one (strip, group, head) unit at a time, so only 1-2 psy banks
    # are ever live) + one filler chunk. PV lags its group's last exp by
    # >= 2 ticks so the PE FIFO never blocks on the scalar engine.
    fillers = []
    pa_gens = {}

    def fill_one():
        while fillers:
            g = fillers.pop(0)
            try:
                next(g)
            except StopIteration:
                continue
            fillers.append(g)
            return True
        return False

    yaccs = {}

    def make_unit(s, gi, grp, h, es_grp):
        """Returns list of thunks: 8 (or 4) PV matmuls then the yacc flush."""
        psy_box = {}

        def mm(u, i):
            def run():
                if u == 0:
                    psy_box["t"] = ps_y.tile([65, SW], F32,
                                             name=f"ps_y_{s}_{gi}_{h}", tag="psy")
                psy = psy_box["t"]
                es = es_grp[i]
                o = i - 4 * s
                last = len(grp) - 1
                if k_diag and o > 0:
                    nc.tensor.matmul(psy[:, 128 * o:], (vtm[h][:, i, :]),
                                     (es[:, h, 128 * o:]),
                                     start=False, stop=(u == last))
                else:
                    nc.tensor.matmul(psy, (vtm[h][:, i, :]), (es[:, h, :]),
                                     start=(u == 0), stop=(u == last))
            return run

        def flush():
            psy = psy_box["t"]
            if gi == 0:
                nc.vector.tensor_copy(yaccs[s][h], psy)
            else:
                nc.vector.tensor_add(yaccs[s][h], yaccs[s][h], psy)

        thunks = [mm(u, i) for u, i in enumerate(grp)]
        thunks.append(flush)
        return thunks

    # PV work queue: per tick pop up to 3 thunks whose eligibility tick passed
    pvq = []          # list of (eligible_tick, thunk)

    def pump_pv(tick, n=3):
        done = 0
        while pvq and done < n:
            et, th = pvq[0]
            if et > tick:
                break
            pvq.pop(0)
            th()
            if th.__name__ != "flush":
                done += 1

    def run_pipeline():
        tick = 0
        for s in range(NS):
            nkt = 4 * s + 4
            qq, tmp = qq_tiles[s]
            qq2 = tmp[0:64, :]
            yaccs[s] = [ya_pool.tile([65, SW], F32, name=f"yacc_{s}_{h}",
                                     tag=f"yacc{h}") for h in range(HPC)]
            if s + 2 < NS:
                phase_a_dma(s + 2)
            if s + 1 < NS:
                g = phase_a_proj(s + 1)
                pa_gens[s + 1] = g
                fillers.append(g)
                fillers.append(phase_a_v(s + 1))
            # ensure this strip's projections are fully issued
            g = pa_gens.get(s)
            if g is not None:
                for _ in g:
                    pass

            groups = [list(range(gg, min(gg + KG, nkt)))
                      for gg in range(0, nkt, KG)]
            es_grp = {}
            for gi, grp in enumerate(groups):
                for u, i in enumerate(grp):
                    pss = ps_s.tile([128, 3, SW], F32,
                                    name=f"ps_s_{s}_{i}", tag="S")
                    st = KK[i // 4]
                    sl = slice((i % 4) * 128, (i % 4) * 128 + 128)
                    nc.tensor.matmul(pss[:, 0, :], (st[0:64, sl]),
                                     (qq[0:64, :]), start=True, stop=True)
                    nc.tensor.matmul(pss[:, 1, :], (st[64:128, sl]),
                                     (qq[64:128, :]), start=True, stop=True)
                    nc.tensor.matmul(pss[:, 2, :], (K2c[i // 4][:, sl]),
                                     (qq2), start=True, stop=True)
                    es = es_pool.tile([128, 3, SW], BF16,
                                      name=f"es_{s}_{i}", tag="es")
                    o = i - 4 * s
                    if o < 0:
                        nc.scalar.activation(es, pss, EXP, scale=0.125)
                    else:
                        nc.scalar.activation(es[:, :, 128 * o:],
                                             pss[:, :, 128 * o:],
                                             EXP, scale=0.125)
                        for h in range(HPC):
                            blk = es[:, h, 128 * o:128 * (o + 1)]
                            nc.gpsimd.affine_select(
                                out=blk, in_=blk,
                                compare_op=mybir.AluOpType.is_ge, fill=0.0,
                                base=0, pattern=[[1, 128]],
                                channel_multiplier=-1)
                        if not k_diag and o > 0:
                            nc.gpsimd.memset(es[:, :, 0:128 * o], 0.0)
                    es_grp[i] = es
                    pump_pv(tick)
                    fill_one()
                    if s <= 2:
                        fill_one()
                    tick += 1
                # group's exps all issued: enqueue its 3 PV units
                et = tick + 1
                for h in range(HPC):
                    for th in make_unit(s, gi, grp, h, dict(es_grp)):
                        pvq.append((et, th))
            # strip done: schedule epilogue after its last units complete
            if s > 0:
                fillers.append(epilogue(s - 1, yaccs[s - 1]))
        # drain
        while pvq:
            et, th = pvq.pop(0)
            th()
            fill_one()
        for _ in epilogue(NS - 1, yaccs[NS - 1]):
            pass
        while fill_one():
            pass

    # ---- strip epilogue: normalize + output projection (deferred) ----
    def epilogue(s, yacc):
        # gather the 3 l-rows onto partitions 0:3; reciprocal in 4 chunks
        # (a single [3,512] reciprocal is 3.3us and blocks the in-order DVE
        # queue, stalling the next strip's qq/KK copies); bounce through
        # DRAM to broadcast across partitions 0:64 per head.
        lrow = rl_pool.tile([3, SW], F32, name=f"lrow_{s}", tag="lrow")
        for h in range(HPC):
            nc.gpsimd.dma_start(lrow[h:h + 1, :], yacc[h][64:65, :])
        yield
        for ch in range(4):
            nc.vector.reciprocal(lrow[:, ch * 128:(ch + 1) * 128],
                                 lrow[:, ch * 128:(ch + 1) * 128])
            yield
        nc.gpsimd.dma_start(scratch_d[s, :, :], lrow)
        yield
        rbs = []
        for h in range(HPC):
            rb = rl_pool.tile([64, SW], F32, name=f"rlb_{s}_{h}", tag=f"rlb{h}")
            nc.gpsimd.dma_start(
                rb, scratch_d[s, h, :].unsqueeze(0).to_broadcast((64, SW)))
            rbs.append(rb)
        yield

        # normalized, stacked Y^T: ya[0:64] = h0, ya[64:128] = h1 (DMA shift)
        ya = yst_pool.tile([128, SW], BF16, name=f"ya_{s}", tag="ya")
        y2 = yst_pool.tile([64, SW], BF16, name=f"y2_{s}", tag="y2")
        ytmp = yst_pool.tile([64, SW], BF16, name=f"ytmp_{s}", tag="ytmp")
        nc.vector.tensor_mul(ya[0:64, :], yacc[0][0:64, :], rbs[0])
        yield
        nc.vector.tensor_mul(ytmp, yacc[1][0:64, :], rbs[1])
        nc.vector.tensor_mul(y2, yacc[2][0:64, :], rbs[2])
        nc.gpsimd.dma_start(ya[64:128, :], ytmp)
        yield

        # out projection per 128-q tile: out = ya.T @ woA + y2.T @ woB
        for qt in range(4):
            pso = ps_s.tile([128, C], F32, name=f"ps_o_{s}_{qt}", tag="S")
            qsl = slice(qt * 128, (qt + 1) * 128)
            for (n0, n1) in ((0, 512), (512, 768)):
                nc.tensor.matmul(pso[:, n0:n1], (ya[:, qsl]),
                                 (woA[:, n0:n1]), start=True, stop=False)
                nc.tensor.matmul(pso[:, n0:n1], (y2[:, qsl]),
                                 (woB[:, n0:n1]), start=False, stop=True)
            osb = out_pool.tile([128, C], F32, name=f"osb_{s}_{qt}", tag="osb")
            nc.vector.tensor_copy(osb, pso)
            nc.sync.dma_start(y_d[s * SW + qt * 128: s * SW + (qt + 1) * 128, :],
                              osb)
            if qt < 3:
                yield

    for _ in phase_a_proj(0):
        pass
    for _ in phase_a_v(0):
        pass
    phase_a_dma(1)
    run_pipeline()


_PROGRAM_CACHE = {}


def _get_program():
    if "nc" not in _PROGRAM_CACHE:
        _PROGRAM_CACHE["nc"] = build_program()
    return _PROGRAM_CACHE["nc"]


def make_in_maps(x, w_qkv, w_out):
    x = np.asarray(x, dtype=np.float32)
    w_qkv = np.asarray(w_qkv, dtype=np.float32)
    w_out = np.asarray(w_out, dtype=np.float32)
    in_maps = []
    for c in range(8):
        b, g = c // 4, c % 4
        base = 192 * g
        q01 = w_qkv[:, base:base + 128]
        q2 = w_qkv[:, base + 128:base + 192]
        k01 = w_qkv[:, 768 + base:768 + base + 128]
        k2 = w_qkv[:, 768 + base + 128:768 + base + 192]
        wqk = np.concatenate([q01, k01, q2, k2], axis=1)
        wv = w_qkv[:, 1536 + base:1536 + base + 192]
        bf = ml_dtypes.bfloat16
        in_maps.append({
            "xT": np.ascontiguousarray(x[b].T.astype(bf)),
            "wqk": np.ascontiguousarray(wqk.astype(bf)),
            "wv": np.ascontiguousarray(wv.astype(bf)),
            "woA": np.ascontiguousarray(w_out[base:base + 128].astype(bf)),
            "woB": np.ascontiguousarray(w_out[base + 128:base + 192].astype(bf)),
        })
    return in_maps


def kernel(x, w_qkv, w_out, trace=False):
    nc = _get_program()
    in_maps = make_in_maps(x, w_qkv, w_out)
    res = run_bass_kernel_spmd(nc, in_maps, list(range(8)), trace=trace)
    out = np.zeros((B, T, C), dtype=np.float32)
    for c in range(8):
        out[c // 4] += res.results[c]["y"]
    kernel.last_result = res
    return out


# revision 16
# speedup vs baseline: 1.0602x; 1.0602x over previous
"""Causal self-attention (B=2, T=4096, C=768, H=12, D=64) on 8 trn2 cores.

Sharding: core c handles batch b = c//4 and heads [3g, 3g+3), g = c%4.
Each core computes a (4096, 768) partial of y = attn_out @ w_out restricted
to its 3 heads' rows of w_out; the host sums the 4 partials per batch.

v1 layout (vs baseline): x arrives host-transposed (xT [C, T]) so no PE
transposes are needed; V is projected token-major directly (stationary =
xT chunk, moving = w_v), QK uses feature-major Q^T/K^T from 3 projection
slots [q0|q1], [k0|k1], [q2|k2] (k2 partition-shifted via SBUF DMA).
Causal masking touches only the [128,128] triangle block per diagonal
k-tile (gpsimd affine_select); the fully-masked columns are skipped by
column-restricted exp + PV accumulation. PV of group g is issued after
the QK+exp of group g+1 so the scalar engine (exp is the global floor,
~1.57us per k-tile) never starves.

Math per head (no max-subtraction softmax; scores are O(8) so exp is safe):
  S^T[k, q] = (K Q^T)[k, q] / 8     computed k-on-partitions (f32r matmuls)
  E = exp(S^T) * causal_mask
  [Y^T; l] = [V | 1]^T E            PV matmul with a ones column -> row 64 = l
  out += (Y^T / l).T @ W_o[head rows]
"""

import os
import numpy as np
import ml_dtypes
from contextlib import ExitStack

import concourse.bass as bass
import concourse.tile as tile
from concourse import bacc, mybir
from concourse.bass_utils import run_bass_kernel_spmd

F32 = mybir.dt.float32
BF16 = mybir.dt.bfloat16

B, T, C, H, D = 2, 4096, 768, 12, 64
HPC = 3            # heads per core
NS = 8             # strips
SW = 512           # strip width (q)
KT = 128           # k tile
NKT = T // KT      # 32 k tiles
KG = 8             # k tiles per PV accumulation group





def build_program():
    nc = bacc.Bacc("TRN2", target_bir_lowering=False, debug=False, num_devices=8)

    x_d = nc.dram_tensor("xT", [C, T], BF16, kind="ExternalInput").ap()
    wqk_d = nc.dram_tensor("wqk", [C, 384], BF16, kind="ExternalInput").ap()
    wv_d = nc.dram_tensor("wv", [C, 192], BF16, kind="ExternalInput").ap()
    woA_d = nc.dram_tensor("woA", [128, C], BF16, kind="ExternalInput").ap()
    woB_d = nc.dram_tensor("woB", [64, C], BF16, kind="ExternalInput").ap()
    y_d = nc.dram_tensor("y", [T, C], F32, kind="ExternalOutput").ap()

    with tile.TileContext(nc) as tc, ExitStack() as ctx:
        kernel_body(tc, ctx, x_d, wqk_d, wv_d, woA_d, woB_d, y_d)
    nc.compile()
    return nc


def kernel_body(tc, ctx, x_d, wqk_d, wv_d, woA_d, woB_d, y_d):
    nc = tc.nc
    EXP = mybir.ActivationFunctionType.Exp
    k_diag = int(os.environ.get("KDIAG", "1"))   # col-restricted diag PV
    k_pb = int(os.environ.get("KPB", "0"))       # gpsimd partition_broadcast
    k_rf = int(os.environ.get("KRF", "0"))       # reciprocal_approx_fast
    k_warm = int(os.environ.get("KWARM", "1"))   # PE warmup matmuls
    dram_pool = ctx.enter_context(tc.tile_pool(name="dram", bufs=1, space="DRAM"))
    scratch_d = dram_pool.tile([NS, HPC, SW], F32, name="scratch")

    singles = ctx.enter_context(tc.tile_pool(name="singles", bufs=1))
    xt_pool = ctx.enter_context(tc.tile_pool(name="xt_pool", bufs=4))
    qq_pool = ctx.enter_context(tc.tile_pool(name="qq_pool", bufs=3))
    es_pool = ctx.enter_context(tc.tile_pool(name="es_pool", bufs=20))
    ya_pool = ctx.enter_context(tc.tile_pool(name="ya_pool", bufs=2))
    rl_pool = ctx.enter_context(tc.tile_pool(name="rl_pool", bufs=2))
    yst_pool = ctx.enter_context(tc.tile_pool(name="yst_pool", bufs=2))
    out_pool = ctx.enter_context(tc.tile_pool(name="out_pool", bufs=2))
    ps_s = ctx.enter_context(tc.tile_pool(name="ps_s", bufs=2, space="PSUM"))
    ps_y = ctx.enter_context(tc.tile_pool(name="ps_y", bufs=2, space="PSUM"))

    # ---- PE warmup: junk matmuls during the initial DMA wait keep HAM hot ----
    junk = singles.tile([128, 128], BF16)
    nc.vector.memset(junk, 0.015625)
    if k_warm:
        psj = ps_y.tile([128, SW], F32, name="ps_warm", tag="psy")
        for w in range(48):
            nc.tensor.matmul(psj[:, 0:128], (junk), (junk),
                             start=True, stop=True)

    # ---- weights (xT strip 0 + wqk first: they gate the first QK) ----
    xT_tiles = [None] * NS
    xt0 = []
    for kc in range(6):
        xt = xt_pool.tile([128, SW], BF16, name=f"xT_0_{kc}", tag=f"xT{kc}")
        nc.sync.dma_start(xt, x_d[kc * 128:(kc + 1) * 128, 0:SW])
        xt0.append(xt)
    xT_tiles[0] = xt0
    wqk_sb = []
    for kc in range(6):
        wt = singles.tile([128, 384], BF16, name=f"wqk_sb{kc}")
        nc.sync.dma_start(wt, wqk_d[kc * 128:(kc + 1) * 128, :])
        wqk_sb.append(wt)
    wv_sb = []
    for kc in range(6):
        wt = singles.tile([128, 192], BF16, name=f"wv_sb{kc}")
        nc.sync.dma_start(wt, wv_d[kc * 128:(kc + 1) * 128, :])
        wv_sb.append(wt)
    woA = singles.tile([128, C], BF16)
    nc.sync.dma_start(woA, woA_d)
    woB = singles.tile([64, C], BF16)
    nc.sync.dma_start(woB, woB_d)

    # resident K storage: KK[s] = [k0|k1] feature-major, K2c[s] = k2 at p0:64
    KK = [singles.tile([128, SW], BF16, name=f"KK{s}") for s in range(NS)]
    K2c = [singles.tile([64, SW], BF16, name=f"K2c{s}") for s in range(NS)]

    # token-major V with ones column per head, all 32 k-tiles
    vtm = [singles.tile([128, NKT, D + 1], BF16, name=f"vtm{h}") for h in range(HPC)]
    ones_col = singles.tile([128, NKT], BF16)
    nc.vector.memset(ones_col, 1.0)
    for h in range(HPC):
        nc.vector.tensor_copy(vtm[h][:, :, D:D + 1], ones_col.unsqueeze(2))

    qq_tiles = [None] * NS

    # ---------------- Phase A for one strip (chunk generator) ----------------
    def phase_a_dma(s):
        xT = []
        for kc in range(6):
            xt = xt_pool.tile([128, SW], BF16, name=f"xT_{s}_{kc}", tag=f"xT{kc}")
            nc.sync.dma_start(xt, x_d[kc * 128:(kc + 1) * 128,
                                      s * SW:(s + 1) * SW])
            xT.append(xt)
        xT_tiles[s] = xT

    def phase_a_proj(s):
        xT = xT_tiles[s]
        # projection slots: [q0|q1], [k0|k1], [q2|k2] -- each slot gets its
        # own 1-bank psum tile from the psy ring so the QK double-buffer
        # ("S" ring) is never starved by projection work
        dests = []
        for u in range(3):
            psp = ps_y.tile([128, SW], F32, name=f"ps_pj_{s}_{u}", tag="psy")
            for kc in range(6):
                nc.tensor.matmul(psp,
                                 (wqk_sb[kc][:, u * 128:(u + 1) * 128]),
                                 (xT[kc]), start=(kc == 0), stop=(kc == 5))
            if u == 0:
                qq = qq_pool.tile([128, SW], BF16, name=f"qq_{s}", tag="qq")
                nc.vector.tensor_copy(qq, psp)
            elif u == 1:
                nc.vector.tensor_copy(KK[s], psp)
            else:
                tmp = qq_pool.tile([128, SW], BF16, name=f"q2k2_{s}",
                                   tag="q2k2")
                nc.vector.tensor_copy(tmp, psp)
                qq_tiles[s] = (qq, tmp)
                # k2 partition shift p64:128 -> p0:64 (SBUF->SBUF DMA)
                nc.gpsimd.dma_start(K2c[s], tmp[64:128, :])
            yield

    def phase_a_v(s):
        # V token-major: stationary = xT chunk slice, moving = w_v [128, 192]
        # (pure filler: vtm k-tiles of strip s are first read by strip s's
        # diagonal PV unit, which issues at the end of strip s)
        xT = xT_tiles[s]
        for tt in range(4):
            psv = ps_y.tile([128, 192], F32, name=f"ps_v_{s}_{tt}", tag="psy")
            for kc in range(6):
                nc.tensor.matmul(psv,
                                 (xT[kc][:, tt * 128:(tt + 1) * 128]),
                                 (wv_sb[kc]), start=(kc == 0), stop=(kc == 5))
            kt = 4 * s + tt
            for h in range(HPC):
                nc.vector.tensor_copy(vtm[h][:, kt, 0:D],
                                      psv[:, h * 64:(h + 1) * 64])
            yield

    # -------- Phase B: one continuous pipeline over all 144 k-tiles --------
    # Per tick (one k-tile): QK triplet + exp + ~3 PV matmuls from the unit
    # queue (one (strip, group, head) unit at a time, so only 1-2 psy banks
    # are ever live) + one filler chunk. PV lags its group's last exp by
    # >= 2 ticks so the PE FIFO never blocks on the scalar engine.
    fillers = []
    pa_gens = {}

    def fill_one():
        while fillers:
            g = fillers.pop(0)
            try:
                next(g)
            except StopIteration:
                continue
            fillers.append(g)
            return True
        return False

    yaccs = {}

    def make_unit(s, gi, grp, h, es_grp):
        """Returns list of thunks: 8 (or 4) PV matmuls then the yacc flush."""
        psy_box = {}

        def mm(u, i):
            def run():
                if u == 0:
                    psy_box["t"] = ps_y.tile([65, SW], F32,
                                             name=f"ps_y_{s}_{gi}_{h}", tag="psy")
                psy = psy_box["t"]
                es = es_grp[i]
                o = i - 4 * s
                last = len(grp) - 1
                if k_diag and o > 0:
                    nc.tensor.matmul(psy[:, 128 * o:], (vtm[h][:, i, :]),
                                     (es[:, h, 128 * o:]),
                                     start=False, stop=(u == last))
                else:
                    nc.tensor.matmul(psy, (vtm[h][:, i, :]), (es[:, h, :]),
                                     start=(u == 0), stop=(u == last))
            return run

        def flush():
            psy = psy_box["t"]
            if gi == 0:
                nc.vector.tensor_copy(yaccs[s][h], psy)
            else:
                nc.vector.tensor_add(yaccs[s][h], yaccs[s][h], psy)

        thunks = [mm(u, i) for u, i in enumerate(grp)]
        thunks.append(flush)
        return thunks

    # PV work queue: per tick pop up to 3 thunks whose eligibility tick passed
    pvq = []          # list of (eligible_tick, thunk)

    def pump_pv(tick, n=3):
        done = 0
        while pvq and done < n:
            et, th = pvq[0]
            if et > tick:
                break
            pvq.pop(0)
            th()
            if th.__name__ != "flush":
                done += 1

    def run_pipeline():
        tick = 0
        for s in range(NS):
            nkt = 4 * s + 4
            if s == 0:
                phase_a_dma(2)
            if s + 3 < NS:
                phase_a_dma(s + 3)
            for sn in ([1, 2] if s == 0 else ([s + 2] if s + 2 < NS else [])):
                g = phase_a_proj(sn)
                pa_gens[sn] = g
                fillers.append(g)
                fillers.append(phase_a_v(sn))
            # ensure this strip's projections are fully issued
            g = pa_gens.get(s)
            if g is not None:
                for _ in g:
                    pass
            qq, tmp = qq_tiles[s]
            qq2 = tmp[0:64, :]
            yaccs[s] = [ya_pool.tile([65, SW], F32, name=f"yacc_{s}_{h}",
                                     tag=f"yacc{h}") for h in range(HPC)]

            groups = [list(range(gg, min(gg + KG, nkt)))
                      for gg in range(0, nkt, KG)]
            es_grp = {}
            for gi, grp in enumerate(groups):
                for u, i in enumerate(grp):
                    pss = ps_s.tile([128, 3, SW], F32,
                                    name=f"ps_s_{s}_{i}", tag="S")
                    st = KK[i // 4]
                    sl = slice((i % 4) * 128, (i % 4) * 128 + 128)
                    nc.tensor.matmul(pss[:, 0, :], (st[0:64, sl]),
                                     (qq[0:64, :]), start=True, stop=True)
                    nc.tensor.matmul(pss[:, 1, :], (st[64:128, sl]),
                                     (qq[64:128, :]), start=True, stop=True)
                    nc.tensor.matmul(pss[:, 2, :], (K2c[i // 4][:, sl]),
                                     (qq2), start=True, stop=True)
                    es = es_pool.tile([128, 3, SW], BF16,
                                      name=f"es_{s}_{i}", tag="es")
                    o = i - 4 * s
                    if o < 0:
                        nc.scalar.activation(es, pss, EXP, scale=0.125)
                    else:
                        nc.scalar.activation(es[:, :, 128 * o:],
                                             pss[:, :, 128 * o:],
                                             EXP, scale=0.125)
                        for h in range(HPC):
                            blk = es[:, h, 128 * o:128 * (o + 1)]
                            nc.gpsimd.affine_select(
                                out=blk, in_=blk,
                                compare_op=mybir.AluOpType.is_ge, fill=0.0,
                                base=0, pattern=[[1, 128]],
                                channel_multiplier=-1)
                        if not k_diag and o > 0:
                            nc.gpsimd.memset(es[:, :, 0:128 * o], 0.0)
                    es_grp[i] = es
                    pump_pv(tick)
                    fill_one()
                    if s <= 2:
                        fill_one()
                    tick += 1
                # group's exps all issued: enqueue its 3 PV units
                et = tick + 1
                for h in range(HPC):
                    for th in make_unit(s, gi, grp, h, dict(es_grp)):
                        pvq.append((et, th))
            # strip done: schedule epilogue after its last units complete
            if s > 0:
                fillers.append(epilogue(s - 1, yaccs[s - 1]))
        # drain
        while pvq:
            et, th = pvq.pop(0)
            th()
            fill_one()
        for _ in epilogue(NS - 1, yaccs[NS - 1]):
            pass
        while fill_one():
            pass

    # ---- strip epilogue: normalize + output projection (deferred) ----
    def epilogue(s, yacc):
        # gather the 3 l-rows onto partitions 0:3; reciprocal in 4 chunks
        # (a single [3,512] reciprocal is 3.3us and blocks the in-order DVE
        # queue, stalling the next strip's qq/KK copies); bounce through
        # DRAM to broadcast across partitions 0:64 per head.
        lrow = rl_pool.tile([3, SW], F32, name=f"lrow_{s}", tag="lrow")
        for h in range(HPC):
            nc.gpsimd.dma_start(lrow[h:h + 1, :], yacc[h][64:65, :])
        yield
        for ch in range(4):
            nc.vector.reciprocal(lrow[:, ch * 128:(ch + 1) * 128],
                                 lrow[:, ch * 128:(ch + 1) * 128])
            yield
        nc.gpsimd.dma_start(scratch_d[s, :, :], lrow)
        yield
        rbs = []
        for h in range(HPC):
            rb = rl_pool.tile([64, SW], F32, name=f"rlb_{s}_{h}", tag=f"rlb{h}")
            nc.gpsimd.dma_start(
                rb, scratch_d[s, h, :].unsqueeze(0).to_broadcast((64, SW)))
            rbs.append(rb)
        yield

        # normalized, stacked Y^T: ya[0:64] = h0, ya[64:128] = h1 (DMA shift)
        ya = yst_pool.tile([128, SW], BF16, name=f"ya_{s}", tag="ya")
        y2 = yst_pool.tile([64, SW], BF16, name=f"y2_{s}", tag="y2")
        ytmp = yst_pool.tile([64, SW], BF16, name=f"ytmp_{s}", tag="ytmp")
        nc.vector.tensor_mul(ya[0:64, :], yacc[0][0:64, :], rbs[0])
        yield
        nc.vector.tensor_mul(ytmp, yacc[1][0:64, :], rbs[1])
        nc.vector.tensor_mul(y2, yacc[2][0:64, :], rbs[2])
        nc.gpsimd.dma_start(ya[64:128, :], ytmp)
        yield

        # out projection per 128-q tile: out = ya.T @ woA + y2.T @ woB
        for qt in range(4):
            qsl = slice(qt * 128, (qt + 1) * 128)
            osb = out_pool.tile([128, C], F32, name=f"osb_{s}_{qt}", tag="osb")
            for (n0, n1) in ((0, 512), (512, 768)):
                pso = ps_y.tile([128, n1 - n0], F32,
                                name=f"ps_o_{s}_{qt}_{n0}", tag="psy")
                nc.tensor.matmul(pso, (ya[:, qsl]),
                                 (woA[:, n0:n1]), start=True, stop=False)
                nc.tensor.matmul(pso, (y2[:, qsl]),
                                 (woB[:, n0:n1]), start=False, stop=True)
                nc.vector.tensor_copy(osb[:, n0:n1], pso)
            nc.sync.dma_start(y_d[s * SW + qt * 128: s * SW + (qt + 1) * 128, :],
                              osb)
            if qt < 3:
                yield

    for _ in phase_a_proj(0):
        pass
    for _ in phase_a_v(0):
        pass
    phase_a_dma(1)
    run_pipeline()


_PROGRAM_CACHE = {}


def _get_program():
    if "nc" not in _PROGRAM_CACHE:
        _PROGRAM_CACHE["nc"] = build_program()
    return _PROGRAM_CACHE["nc"]


def make_in_maps(x, w_qkv, w_out):
    x = np.asarray(x, dtype=np.float32)
    w_qkv = np.asarray(w_qkv, dtype=np.float32)
    w_out = np.asarray(w_out, dtype=np.float32)
    in_maps = []
    for c in range(8):
        b, g = c // 4, c % 4
        base = 192 * g
        q01 = w_qkv[:, base:base + 128]
        q2 = w_qkv[:, base + 128:base + 192]
        k01 = w_qkv[:, 768 + base:768 + base + 128]
        k2 = w_qkv[:, 768 + base + 128:768 + base + 192]
        wqk = np.concatenate([q01, k01, q2, k2], axis=1)
        wv = w_qkv[:, 1536 + base:1536 + base + 192]
        bf = ml_dtypes.bfloat16
        in_maps.append({
            "xT": np.ascontiguousarray(x[b].T.astype(bf)),
            "wqk": np.ascontiguousarray(wqk.astype(bf)),
            "wv": np.ascontiguousarray(wv.astype(bf)),
            "woA": np.ascontiguousarray(w_out[base:base + 128].astype(bf)),
            "woB": np.ascontiguousarray(w_out[base + 128:base + 192].astype(bf)),
        })
    return in_maps


def kernel(x, w_qkv, w_out, trace=False):
    nc = _get_program()
    in_maps = make_in_maps(x, w_qkv, w_out)
    res = run_bass_kernel_spmd(nc, in_maps, list(range(8)), trace=trace)
    out = np.zeros((B, T, C), dtype=np.float32)
    for c in range(8):
        out[c // 4] += res.results[c]["y"]
    kernel.last_result = res
    return out


# revision 17
# speedup vs baseline: 1.0803x; 1.0189x over previous
"""Causal self-attention (B=2, T=4096, C=768, H=12, D=64) on 8 trn2 cores.

Sharding: core c handles batch b = c//4 and heads [3g, 3g+3), g = c%4.
Each core computes a (4096, 768) partial of y = attn_out @ w_out restricted
to its 3 heads' rows of w_out; the host sums the 4 partials per batch.

v1 layout (vs baseline): x arrives host-transposed (xT [C, T]) so no PE
transposes are needed; V is projected token-major directly (stationary =
xT chunk, moving = w_v), QK uses feature-major Q^T/K^T from 3 projection
slots [q0|q1], [k0|k1], [q2|k2] (k2 partition-shifted via SBUF DMA).
Causal masking touches only the [128,128] triangle block per diagonal
k-tile (gpsimd affine_select); the fully-masked columns are skipped by
column-restricted exp + PV accumulation. PV of group g is issued after
the QK+exp of group g+1 so the scalar engine (exp is the global floor,
~1.57us per k-tile) never starves.

Math per head (no max-subtraction softmax; scores are O(8) so exp is safe):
  S^T[k, q] = (K Q^T)[k, q] / 8     computed k-on-partitions (f32r matmuls)
  E = exp(S^T) * causal_mask
  [Y^T; l] = [V | 1]^T E            PV matmul with a ones column -> row 64 = l
  out += (Y^T / l).T @ W_o[head rows]
"""

import os
import numpy as np
import ml_dtypes
from contextlib import ExitStack

import concourse.bass as bass
import concourse.tile as tile
from concourse import bacc, mybir
from concourse.bass_utils import run_bass_kernel_spmd

F32 = mybir.dt.float32
BF16 = mybir.dt.bfloat16

B, T, C, H, D = 2, 4096, 768, 12, 64
HPC = 3            # heads per core
NS = 8             # strips
SW = 512           # strip width (q)
KT = 128           # k tile
NKT = T // KT      # 32 k tiles
KG = 8             # k tiles per PV accumulation group





def build_program():
    nc = bacc.Bacc("TRN2", target_bir_lowering=False, debug=False, num_devices=8)

    x_d = nc.dram_tensor("xT", [C, T], BF16, kind="ExternalInput").ap()
    wqk_d = nc.dram_tensor("wqk", [C, 384], BF16, kind="ExternalInput").ap()
    wv_d = nc.dram_tensor("wv", [C, 192], BF16, kind="ExternalInput").ap()
    woA_d = nc.dram_tensor("woA", [128, C], BF16, kind="ExternalInput").ap()
    woB_d = nc.dram_tensor("woB", [64, C], BF16, kind="ExternalInput").ap()
    y_d = nc.dram_tensor("y", [T, C], F32, kind="ExternalOutput").ap()

    with tile.TileContext(nc) as tc, ExitStack() as ctx:
        kernel_body(tc, ctx, x_d, wqk_d, wv_d, woA_d, woB_d, y_d)
    nc.compile()
    return nc


def kernel_body(tc, ctx, x_d, wqk_d, wv_d, woA_d, woB_d, y_d):
    nc = tc.nc
    EXP = mybir.ActivationFunctionType.Exp
    k_diag = int(os.environ.get("KDIAG", "1"))   # col-restricted diag PV
    k_pb = int(os.environ.get("KPB", "0"))       # gpsimd partition_broadcast
    k_rf = int(os.environ.get("KRF", "0"))       # reciprocal_approx_fast
    k_warm = int(os.environ.get("KWARM", "1"))   # PE warmup matmuls
    dram_pool = ctx.enter_context(tc.tile_pool(name="dram", bufs=1, space="DRAM"))
    scratch_d = dram_pool.tile([NS, HPC, SW], F32, name="scratch")

    singles = ctx.enter_context(tc.tile_pool(name="singles", bufs=1))
    xt_pool = ctx.enter_context(tc.tile_pool(name="xt_pool", bufs=4))
    qq_pool = ctx.enter_context(tc.tile_pool(name="qq_pool", bufs=3))
    es_pool = ctx.enter_context(tc.tile_pool(name="es_pool", bufs=20))
    ya_pool = ctx.enter_context(tc.tile_pool(name="ya_pool", bufs=2))
    rl_pool = ctx.enter_context(tc.tile_pool(name="rl_pool", bufs=2))
    yst_pool = ctx.enter_context(tc.tile_pool(name="yst_pool", bufs=2))
    out_pool = ctx.enter_context(tc.tile_pool(name="out_pool", bufs=2))
    ps_s = ctx.enter_context(tc.tile_pool(name="ps_s", bufs=2, space="PSUM"))
    ps_y = ctx.enter_context(tc.tile_pool(name="ps_y", bufs=2, space="PSUM"))

    # ---- PE warmup: junk matmuls during the initial DMA wait keep HAM hot ----
    junk = singles.tile([128, 128], BF16)
    nc.vector.memset(junk, 0.015625)
    if k_warm:
        psj = ps_y.tile([128, SW], F32, name="ps_warm", tag="psy")
        for w in range(48):
            nc.tensor.matmul(psj[:, 0:128], (junk), (junk),
                             start=True, stop=True)

    # ---- weights (xT strip 0 + wqk first: they gate the first QK) ----
    xT_tiles = [None] * NS
    xt0 = []
    for kc in range(6):
        xt = xt_pool.tile([128, SW], BF16, name=f"xT_0_{kc}", tag=f"xT{kc}")
        nc.sync.dma_start(xt, x_d[kc * 128:(kc + 1) * 128, 0:SW])
        xt0.append(xt)
    xT_tiles[0] = xt0
    wqk_sb = []
    for kc in range(6):
        wt = singles.tile([128, 384], BF16, name=f"wqk_sb{kc}")
        nc.sync.dma_start(wt, wqk_d[kc * 128:(kc + 1) * 128, :])
        wqk_sb.append(wt)
    wv_sb = []
    for kc in range(6):
        wt = singles.tile([128, 192], BF16, name=f"wv_sb{kc}")
        nc.sync.dma_start(wt, wv_d[kc * 128:(kc + 1) * 128, :])
        wv_sb.append(wt)
    woA = singles.tile([128, C], BF16)
    nc.sync.dma_start(woA, woA_d)
    woB = singles.tile([64, C], BF16)
    nc.sync.dma_start(woB, woB_d)

    # resident K storage: KK[s] = [k0|k1] feature-major, K2c[s] = k2 at p0:64
    KK = [singles.tile([128, SW], BF16, name=f"KK{s}") for s in range(NS)]
    K2c = [singles.tile([64, SW], BF16, name=f"K2c{s}") for s in range(NS)]

    # token-major V with ones column per head, all 32 k-tiles
    vtm = [singles.tile([128, NKT, D + 1], BF16, name=f"vtm{h}") for h in range(HPC)]
    ones_col = singles.tile([128, NKT], BF16)
    nc.vector.memset(ones_col, 1.0)
    for h in range(HPC):
        nc.vector.tensor_copy(vtm[h][:, :, D:D + 1], ones_col.unsqueeze(2))

    qq_tiles = [None] * NS

    # ---------------- Phase A for one strip (chunk generator) ----------------
    def phase_a_dma(s):
        xT = []
        for kc in range(6):
            xt = xt_pool.tile([128, SW], BF16, name=f"xT_{s}_{kc}", tag=f"xT{kc}")
            nc.sync.dma_start(xt, x_d[kc * 128:(kc + 1) * 128,
                                      s * SW:(s + 1) * SW])
            xT.append(xt)
        xT_tiles[s] = xT

    def phase_a_proj(s):
        xT = xT_tiles[s]
        # projection slots: [q0|q1], [k0|k1], [q2|k2] -- each slot gets its
        # own 1-bank psum tile from the psy ring so the QK double-buffer
        # ("S" ring) is never starved by projection work
        dests = []
        for u in range(3):
            psp = ps_y.tile([128, SW], F32, name=f"ps_pj_{s}_{u}", tag="psy")
            for kc in range(6):
                nc.tensor.matmul(psp,
                                 (wqk_sb[kc][:, u * 128:(u + 1) * 128]),
                                 (xT[kc]), start=(kc == 0), stop=(kc == 5))
            if u == 0:
                qq = qq_pool.tile([128, SW], BF16, name=f"qq_{s}", tag="qq")
                nc.vector.tensor_copy(qq, psp)
            elif u == 1:
                nc.vector.tensor_copy(KK[s], psp)
            else:
                tmp = qq_pool.tile([128, SW], BF16, name=f"q2k2_{s}",
                                   tag="q2k2")
                nc.vector.tensor_copy(tmp, psp)
                qq_tiles[s] = (qq, tmp)
                # k2 partition shift p64:128 -> p0:64 (SBUF->SBUF DMA)
                nc.gpsimd.dma_start(K2c[s], tmp[64:128, :])
            yield

    def phase_a_v(s):
        # V token-major: stationary = xT chunk slice, moving = w_v [128, 192]
        # (pure filler: vtm k-tiles of strip s are first read by strip s's
        # diagonal PV unit, which issues at the end of strip s)
        xT = xT_tiles[s]
        for tt in range(4):
            psv = ps_y.tile([128, 192], F32, name=f"ps_v_{s}_{tt}", tag="psy")
            for kc in range(6):
                nc.tensor.matmul(psv,
                                 (xT[kc][:, tt * 128:(tt + 1) * 128]),
                                 (wv_sb[kc]), start=(kc == 0), stop=(kc == 5))
            kt = 4 * s + tt
            for h in range(HPC):
                nc.vector.tensor_copy(vtm[h][:, kt, 0:D],
                                      psv[:, h * 64:(h + 1) * 64])
            yield

    # -------- Phase B: one continuous pipeline over all 144 k-tiles --------
    # Per tick (one k-tile): QK triplet + exp + ~3 PV matmuls from the unit
    # queue (one (strip, group, head) unit at a time, so only 1-2 psy banks
    # are ever live) + one filler chunk. PV lags its group's last exp by
    # >= 2 ticks so the PE FIFO never blocks on the scalar engine.
    fillers = []
    pa_gens = {}

    def fill_one():
        while fillers:
            g = fillers.pop(0)
            try:
                next(g)
            except StopIteration:
                continue
            fillers.append(g)
            return True
        return False

    yaccs = {}

    def make_unit(s, gi, grp, h, es_grp):
        """Returns list of thunks: 8 (or 4) PV matmuls then the yacc flush."""
        psy_box = {}

        def mm(u, i):
            def run():
                if u == 0:
                    psy_box["t"] = ps_y.tile([65, SW], F32,
                                             name=f"ps_y_{s}_{gi}_{h}", tag="psy")
                psy = psy_box["t"]
                es = es_grp[i]
                o = i - 4 * s
                last = len(grp) - 1
                if k_diag and o > 0:
                    nc.tensor.matmul(psy[:, 128 * o:], (vtm[h][:, i, :]),
                                     (es[:, h, 128 * o:]),
                                     start=False, stop=(u == last))
                else:
                    nc.tensor.matmul(psy, (vtm[h][:, i, :]), (es[:, h, :]),
                                     start=(u == 0), stop=(u == last))
            return run

        def flush():
            psy = psy_box["t"]
            if gi == 0:
                nc.vector.tensor_copy(yaccs[s][h], psy)
            else:
                nc.vector.tensor_add(yaccs[s][h], yaccs[s][h], psy)

        thunks = [mm(u, i) for u, i in enumerate(grp)]
        thunks.append(flush)
        return thunks

    # PV work queue: per tick pop up to 3 thunks whose eligibility tick passed
    pvq = []          # list of (eligible_tick, thunk)

    def pump_pv(tick, n=3):
        done = 0
        while pvq and done < n:
            et, th = pvq[0]
            if et > tick:
                break
            pvq.pop(0)
            th()
            if th.__name__ != "flush":
                done += 1

    def run_pipeline():
        tick = 0
        for s in range(NS):
            nkt = 4 * s + 4
            if s + 2 < NS:
                phase_a_dma(s + 2)
            if s + 1 < NS:
                g = phase_a_proj(s + 1)
                pa_gens[s + 1] = g
                fillers.append(g)
            if s >= 1:
                fillers.append(phase_a_v(s))
            # ensure this strip's projections are fully issued
            g = pa_gens.get(s)
            if g is not None:
                for _ in g:
                    pass
            qq, tmp = qq_tiles[s]
            qq2 = tmp[0:64, :]
            yaccs[s] = [ya_pool.tile([65, SW], F32, name=f"yacc_{s}_{h}",
                                     tag=f"yacc{h}") for h in range(HPC)]

            groups = [list(range(gg, min(gg + KG, nkt)))
                      for gg in range(0, nkt, KG)]
            es_grp = {}
            for gi, grp in enumerate(groups):
                for u, i in enumerate(grp):
                    pss = ps_s.tile([128, 3, SW], F32,
                                    name=f"ps_s_{s}_{i}", tag="S")
                    st = KK[i // 4]
                    sl = slice((i % 4) * 128, (i % 4) * 128 + 128)
                    nc.tensor.matmul(pss[:, 0, :], (st[0:64, sl]),
                                     (qq[0:64, :]), start=True, stop=True)
                    nc.tensor.matmul(pss[:, 1, :], (st[64:128, sl]),
                                     (qq[64:128, :]), start=True, stop=True)
                    nc.tensor.matmul(pss[:, 2, :], (K2c[i // 4][:, sl]),
                                     (qq2), start=True, stop=True)
                    es = es_pool.tile([128, 3, SW], BF16,
                                      name=f"es_{s}_{i}", tag="es")
                    o = i - 4 * s
                    if o < 0:
                        nc.scalar.activation(es, pss, EXP, scale=0.125)
                    else:
                        nc.scalar.activation(es[:, :, 128 * o:],
                                             pss[:, :, 128 * o:],
                                             EXP, scale=0.125)
                        for h in range(HPC):
                            blk = es[:, h, 128 * o:128 * (o + 1)]
                            nc.gpsimd.affine_select(
                                out=blk, in_=blk,
                                compare_op=mybir.AluOpType.is_ge, fill=0.0,
                                base=0, pattern=[[1, 128]],
                                channel_multiplier=-1)
                        if not k_diag and o > 0:
                            nc.gpsimd.memset(es[:, :, 0:128 * o], 0.0)
                    es_grp[i] = es
                    pump_pv(tick)
                    fill_one()
                    if 1 <= s <= 2:
                        fill_one()
                    tick += 1
                # group's exps all issued: enqueue its 3 PV units
                et = tick + 1
                for h in range(HPC):
                    for th in make_unit(s, gi, grp, h, dict(es_grp)):
                        pvq.append((et, th))
            # strip done: schedule epilogue after its last units complete
            if s > 0:
                fillers.append(epilogue(s - 1, yaccs[s - 1]))
        # drain
        while pvq:
            et, th = pvq.pop(0)
            th()
            fill_one()
        for _ in epilogue(NS - 1, yaccs[NS - 1]):
            pass
        while fill_one():
            pass

    # ---- strip epilogue: normalize + output projection (deferred) ----
    def epilogue(s, yacc):
        # gather the 3 l-rows onto partitions 0:3; reciprocal in 4 chunks
        # (a single [3,512] reciprocal is 3.3us and blocks the in-order DVE
        # queue, stalling the next strip's qq/KK copies); bounce through
        # DRAM to broadcast across partitions 0:64 per head.
        lrow = rl_pool.tile([3, SW], F32, name=f"lrow_{s}", tag="lrow")
        for h in range(HPC):
            nc.gpsimd.dma_start(lrow[h:h + 1, :], yacc[h][64:65, :])
        yield
        for ch in range(4):
            nc.vector.reciprocal(lrow[:, ch * 128:(ch + 1) * 128],
                                 lrow[:, ch * 128:(ch + 1) * 128])
            yield
        nc.gpsimd.dma_start(scratch_d[s, :, :], lrow)
        yield
        rbs = []
        for h in range(HPC):
            rb = rl_pool.tile([64, SW], F32, name=f"rlb_{s}_{h}", tag=f"rlb{h}")
            nc.gpsimd.dma_start(
                rb, scratch_d[s, h, :].unsqueeze(0).to_broadcast((64, SW)))
            rbs.append(rb)
        yield

        # normalized, stacked Y^T: ya[0:64] = h0, ya[64:128] = h1 (DMA shift)
        ya = yst_pool.tile([128, SW], BF16, name=f"ya_{s}", tag="ya")
        y2 = yst_pool.tile([64, SW], BF16, name=f"y2_{s}", tag="y2")
        ytmp = yst_pool.tile([64, SW], BF16, name=f"ytmp_{s}", tag="ytmp")
        nc.vector.tensor_mul(ya[0:64, :], yacc[0][0:64, :], rbs[0])
        yield
        nc.vector.tensor_mul(ytmp, yacc[1][0:64, :], rbs[1])
        nc.vector.tensor_mul(y2, yacc[2][0:64, :], rbs[2])
        nc.gpsimd.dma_start(ya[64:128, :], ytmp)
        yield

        # out projection per 128-q tile: out = ya.T @ woA + y2.T @ woB
        for qt in range(4):
            qsl = slice(qt * 128, (qt + 1) * 128)
            osb = out_pool.tile([128, C], F32, name=f"osb_{s}_{qt}", tag="osb")
            for (n0, n1) in ((0, 512), (512, 768)):
                pso = ps_y.tile([128, n1 - n0], F32,
                                name=f"ps_o_{s}_{qt}_{n0}", tag="psy")
                nc.tensor.matmul(pso, (ya[:, qsl]),
                                 (woA[:, n0:n1]), start=True, stop=False)
                nc.tensor.matmul(pso, (y2[:, qsl]),
                                 (woB[:, n0:n1]), start=False, stop=True)
                nc.vector.tensor_copy(osb[:, n0:n1], pso)
            nc.sync.dma_start(y_d[s * SW + qt * 128: s * SW + (qt + 1) * 128, :],
                              osb)
            if qt < 3:
                yield

    for _ in phase_a_proj(0):
        pass
    for _ in phase_a_v(0):
        pass
    phase_a_dma(1)
    run_pipeline()


_PROGRAM_CACHE = {}


def _get_program():
    if "nc" not in _PROGRAM_CACHE:
        _PROGRAM_CACHE["nc"] = build_program()
    return _PROGRAM_CACHE["nc"]


def make_in_maps(x, w_qkv, w_out):
    x = np.asarray(x, dtype=np.float32)
    w_qkv = np.asarray(w_qkv, dtype=np.float32)
    w_out = np.asarray(w_out, dtype=np.float32)
    in_maps = []
    for c in range(8):
        b, g = c // 4, c % 4
        base = 192 * g
        q01 = w_qkv[:, base:base + 128]
        q2 = w_qkv[:, base + 128:base + 192]
        k01 = w_qkv[:, 768 + base:768 + base + 128]
        k2 = w_qkv[:, 768 + base + 128:768 + base + 192]
        wqk = np.concatenate([q01, k01, q2, k2], axis=1)
        wv = w_qkv[:, 1536 + base:1536 + base + 192]
        bf = ml_dtypes.bfloat16
        in_maps.append({
            "xT": np.ascontiguousarray(x[b].T.astype(bf)),
            "wqk": np.ascontiguousarray(wqk.astype(bf)),
            "wv": np.ascontiguousarray(wv.astype(bf)),
            "woA": np.ascontiguousarray(w_out[base:base + 128].astype(bf)),
            "woB": np.ascontiguousarray(w_out[base + 128:base + 192].astype(bf)),
        })
    return in_maps


def kernel(x, w_qkv, w_out, trace=False):
    nc = _get_program()
    in_maps = make_in_maps(x, w_qkv, w_out)
    res = run_bass_kernel_spmd(nc, in_maps, list(range(8)), trace=trace)
    out = np.zeros((B, T, C), dtype=np.float32)
    for c in range(8):
        out[c // 4] += res.results[c]["y"]
    kernel.last_result = res
    return out
